# revision 11
# baseline (speedup 1.0000x reference)
"""Per-sample covariance kernel for Trainium2 (8 NeuronCores, data-parallel).

Problem: X [64, 256, 2048] f32  ->  cov [64, 256, 256] f32 where
    cov[b] = (X[b] - mean_t(X[b])) @ (X[b] - mean_t(X[b]))^T / T

Strategy (per core, 8 samples each):
  cov = G/T - (s/T)(s/T)^T  with  G = X @ X^T,  s = X @ ones.
  - DMA X[b] into SBUF in natural [c, t] layout.
  - PE-transpose to XT tiles [t, c] (fp32 via identity matmul), one
    [128, 257] tile per 128-wide t-chunk; column 256 holds constant 1.0
    so every Gram matmul also produces the row-sums s in its last column.
  - 2 m-blocks x 16 k-chunks accumulating matmuls -> G blocks in PSUM.
    (fp32r wants even free sizes, so tiles carry 258 columns: 256 data +
    a ones column for the sums + one dummy ones column of padding.)
  - Extract s, build the -s/T row via a tiny PE transpose, then one K=1
    matmul per m-block adds -(s_m)(s_n)/T into the open PSUM group.
  - Scale by 1/T on the way out (ACT engine), DMA to DRAM.
"""

import os
import sys
from contextlib import ExitStack

import numpy as np


def _ensure_concourse():
    try:
        import concourse  # noqa: F401
    except ImportError:
        for p in ("/opt/trn_rl_repo", "/root/.axon_site/_ro/trn_rl_repo"):
            if os.path.isdir(p) and p not in sys.path:
                sys.path.insert(0, p)


_ensure_concourse()

import concourse.bass as bass  # noqa: E402,F401
import concourse.tile as tile  # noqa: E402
from concourse import bacc, mybir  # noqa: E402
from concourse.bass_utils import run_bass_kernel_spmd  # noqa: E402
from concourse.masks import make_identity  # noqa: E402

B, C, T = 64, 256, 2048
NCORES = 8
BPC = B // NCORES  # samples per core
P = 128
KCH = T // P  # contraction chunks of 128
CB = C // P  # 128-row blocks of C
F32 = mybir.dt.float32

# matmul operand dtype: float32r streams 1 row/cycle (vs 4 for float32)
MM_DT = getattr(mybir.dt, os.environ.get("COV_MM_DT", "float32r"))


def build_nc(mm_dt=MM_DT):
    nc = bacc.Bacc("TRN2", target_bir_lowering=False, debug=False)
    X = nc.declare_dram_parameter("X", [BPC, C, T], F32, isOutput=False)
    OUT = nc.declare_dram_parameter("OUT", [BPC, C, C], F32, isOutput=True)
    inv_t = 1.0 / T

    with ExitStack() as ctx:
        tc = ctx.enter_context(tile.TileContext(nc))
        singles = ctx.enter_context(tc.tile_pool(name="singles", bufs=1))
        xpool = ctx.enter_context(tc.tile_pool(name="xnat", bufs=3))
        tpsum = ctx.enter_context(tc.tile_pool(name="tpsum", bufs=2, space="PSUM"))
        gpsum = ctx.enter_context(tc.tile_pool(name="gpsum", bufs=2, space="PSUM"))
        spsum = ctx.enter_context(tc.tile_pool(name="spsum", bufs=2, space="PSUM"))
        small = ctx.enter_context(tc.tile_pool(name="small", bufs=4))
        opool = ctx.enter_context(tc.tile_pool(name="opool", bufs=4))

        ident = singles.tile([P, P], F32)
        make_identity(nc, ident)

        # Ping-pong transposed-layout tiles. The ones-column (col C) is
        # written once here and never touched by the transpose copies.
        xts = [
            singles.tile([P, KCH, C + 2], mm_dt, name=f"xt{i}", tag=f"xt{i}")
            for i in range(2)
        ]
        ones = singles.tile([P, 1], F32)
        nc.vector.memset(ones, 1.0)
        for xt in xts:
            # memset can't write float32r; a DVE copy rounds legally.
            nc.vector.tensor_copy(
                out=xt[:, :, C : C + 2], in_=ones.to_broadcast([P, KCH, 2])
            )

        for b in range(BPC):
            xt = xts[b % 2]
            xn = xpool.tile([P, CB, T], F32)
            nc.sync.dma_start(out=xn, in_=X[b].rearrange("(cb p) t -> p cb t", p=P))

            for k in range(KCH):
                pt = tpsum.tile([P, C], F32)
                for cb in range(CB):
                    nc.tensor.matmul(
                        pt[:, cb * P : (cb + 1) * P],
                        xn[:, cb, k * P : (k + 1) * P],
                        ident,
                        is_transpose=True,
                        start=(cb == 0),
                        stop=(cb == CB - 1),
                    )
                nc.vector.tensor_copy(out=xt[:, k, 0:C], in_=pt)

            # Gram accumulation; group stays open for the mean correction.
            psg = [
                gpsum.tile([P, C + 2], F32, name=f"g{mb}", tag=f"g{mb}")
                for mb in range(CB)
            ]
            for mb in range(CB):
                for k in range(KCH):
                    nc.tensor.matmul(
                        psg[mb],
                        xt[:, k, mb * P : (mb + 1) * P],
                        xt[:, k, :],
                        start=(k == 0),
                        stop=(k == KCH - 1),
                    )

            # s -> row layout: copy the two PSUM sum-columns to SBUF and
            # PE-transpose them side by side onto partition 0.
            scol = small.tile([P, CB], F32)
            for mb in range(CB):
                nc.vector.tensor_copy(
                    out=scol[:, mb : mb + 1], in_=psg[mb][:, C : C + 1]
                )
            srow_ps = spsum.tile([1, C], F32)
            for mb in range(CB):
                nc.tensor.matmul(
                    srow_ps[0:1, mb * P : (mb + 1) * P],
                    scol[:, mb : mb + 1],
                    ident,
                    is_transpose=True,
                    start=(mb == 0),
                    stop=(mb == CB - 1),
                )
            srow = small.tile([1, C], mm_dt)
            nsrow = small.tile([1, C], mm_dt)
            nc.scalar.copy(out=srow, in_=srow_ps)
            nc.scalar.mul(out=nsrow, in_=srow_ps, mul=-inv_t)

            # K=1 rank-1 update: psg += (-s/T) s^T. The Gram group is already
            # closed (sim bookkeeping); on HW has_written persists, so
            # start=False still accumulates onto the existing values.
            for mb in range(CB):
                nc.tensor.matmul(
                    psg[mb][:, 0:C],
                    nsrow[0:1, mb * P : (mb + 1) * P],
                    srow,
                    start=False,
                    stop=True,
                    skip_group_check=True,
                )

            for mb in range(CB):
                ot = opool.tile([P, C], F32)
                nc.scalar.mul(out=ot, in_=psg[mb][:, 0:C], mul=inv_t)
                nc.sync.dma_start(out=OUT[b, mb * P : (mb + 1) * P, :], in_=ot)

    nc.compile()
    return nc


def kernel(X: np.ndarray) -> np.ndarray:
    assert X.shape == (B, C, T), X.shape
    X = np.ascontiguousarray(X, dtype=np.float32)
    nc = build_nc()
    in_maps = [{"X": X[i * BPC : (i + 1) * BPC]} for i in range(NCORES)]
    res = run_bass_kernel_spmd(nc, in_maps, core_ids=list(range(NCORES)))
    return np.concatenate([res.results[i]["OUT"] for i in range(NCORES)], axis=0)
